# revision 1
# baseline (speedup 1.0000x reference)
"""Trainium2 Bass kernel for the binarized-conv BasicBlock problem.

Math restructure (exact up to fp32-grade rounding):
  wb = sign(weight)  (+-1 exactly representable in fp16)
  out = clip( A * conv(x, wb) + x + B , -1, 1 )
where
  A[o]     = gamma/sqrt(var+eps) * (1 + w1[o])                (per channel)
  B[o,h,w] = bs*(conv(S1,wb) + w1*conv(S2,wb))[o,h,w] + bb[o] (batch-independent
             shift/edge field, computed on host)
Precision: x is split on host as x = x16 + d16 (both fp16), so the conv is
fp32-accurate: conv(x) = conv(x16) + conv(d16) with exact +-1 weights
contracted as K=128 = [x16 ch; d16 ch] in one matmul stream per image.
The residual +x is applied as x16 via DVE (partition-aligned) plus d16 via
a +1 on the center-tap identity rows.

PE mapping: per image pair, rhsA = [x16 A; d16 A], rhsB = [d16 B; x16 B]
(flipped so x16 B is partition-aligned with img B's psum half).  Each tap is
two K=128, M=64 matmuls on the two column halves of the PE array; redundant
LDWEIGHTS are deduplicated post-legalize so the column halves stream
concurrently with weights held stationary across the 7 row-chunks of a tap.
Sharding: batch 64 -> 8 cores x 8 images.
"""
import sys
for _p in ('/opt/trn_rl_repo',):
    if _p not in sys.path:
        sys.path.insert(0, _p)

import numpy as np
import concourse.bass as bass
import concourse.bacc as bacc
import concourse.tile as tile
import concourse.mybir as mybir
from concourse import bass_utils

BN_EPS = 1e-5
N_CORES = 8
C, H, W = 64, 56, 56
HP, WP = H + 2, W + 2           # padded spatial
NPOS = H * W                    # 3136
PPOS = HP * WP                  # 3364
IMGS = 8                        # images per core
PAIRS = IMGS // 2
CH_ROWS = 8                     # output rows per chunk
NCHUNK = H // CH_ROWS           # 7
CHL = CH_ROWS * W               # 448

f32 = mybir.dt.float32
f16 = mybir.dt.float16
ALU = mybir.AluOpType
AF = mybir.ActivationFunctionType

_CACHE = {}


def _ldw_sig(inst):
    ap = inst.ins[0]
    bap = ap.bass_ap
    return (bap.tensor.name, bap.offset, str(bap.ap), str(inst.tile_position))


def _dedup_ldweights(ordered, enable=True):
    """Drop LDWEIGHTS that reload the exact weights already resident in the
    same PE column group, so matmuls on alternating column halves stream
    back-to-back (a reload between them serializes the halves: the load
    conflicts with the in-flight matmul's row groups)."""
    if not enable:
        return ordered
    for bb, insts in ordered.items():
        last = {}     # col position -> (sig, kept name)
        remap = {}
        keep = []
        pending = None  # dropped ldw awaiting its matmul to absorb deps
        for inst in insts:
            if isinstance(inst, mybir.InstLdweights):
                sig = _ldw_sig(inst)
                col = (inst.tile_position or (0, 0))[1]
                ent = last.get(col)
                if ent is not None and ent[0] == sig:
                    remap[inst.name] = ent[1]
                    pending = inst
                    continue
                last[col] = (sig, inst.name)
            elif isinstance(inst, mybir.InstMatmult) and pending is not None:
                try:
                    inst.merge_dependencies_from(pending)
                except Exception:
                    inst.add_sync_dependencies_from(pending)
                pending = None
            keep.append(inst)
        if remap:
            for inst in keep:
                inst.remap_dependency_names(remap)
        ordered[bb] = keep
    return ordered


def _build_module(repeat=1, hw_loop=0, ablate=(), compile=True, dedup=True):
    nc = bacc.Bacc("TRN2", target_bir_lowering=False, debug=False,
                   enable_asserts=False, num_devices=N_CORES)

    # pre-padded on host: [pair, {A,B}, 128, 58*58] fp16
    xr_d = nc.dram_tensor("xr", [PAIRS, 2, 128, PPOS], f16, kind="ExternalInput").ap()
    w_d = nc.dram_tensor("wt", [128, 9 * 128], f16, kind="ExternalInput").ap()
    a_d = nc.dram_tensor("ascale", [128, 1], f32, kind="ExternalInput").ap()
    b_d = nc.dram_tensor("bfield", [128, NPOS], f32, kind="ExternalInput").ap()
    y_d = nc.dram_tensor("y", [PAIRS, 128, NPOS], f32, kind="ExternalOutput").ap()

    import concourse.tile as tile_mod
    orig_legalize = tile_mod.tile_legalize
    tile_mod.tile_legalize = lambda ordered, nc_, _o=orig_legalize: _dedup_ldweights(
        _o(ordered, nc_), enable=dedup)
    try:
        with tile.TileContext(nc) as tc:
            with tc.tile_pool(name="const", bufs=1) as constp, \
                 tc.tile_pool(name="rhs", bufs=2) as rhsp, \
                 tc.tile_pool(name="eout", bufs=4) as outp, \
                 tc.tile_pool(name="psum", bufs=1, space="PSUM") as psp:
                wt = constp.tile([128, 9 * 128], f16)
                nc.sync.dma_start(wt[:], w_d[:])
                at = constp.tile([128, 1], f32)
                nc.sync.dma_start(at[:], a_d[:])
                bt = constp.tile([128, NPOS], f32)
                nc.sync.dma_start(bt[:], b_d[:])
                two = constp.tile([128, 1], f32)
                nc.vector.memset(two[:], 2.0)

                def _body():
                  for p in [pp for _ in range(repeat) for pp in range(PAIRS)]:
                    ra = rhsp.tile([128, PPOS], f16, tag="ra")
                    rb = rhsp.tile([128, PPOS], f16, tag="rb")
                    if 'dma_in' not in ablate:
                        nc.sync.dma_start(ra[:], xr_d[p, 0])
                        nc.sync.dma_start(rb[:], xr_d[p, 1])
                    ra3 = ra[:].rearrange("p (h w) -> p h w", w=WP)
                    rb3 = rb[:].rearrange("p (h w) -> p h w", w=WP)

                    pss = []
                    for c in range(NCHUNK):
                        ps_c = psp.tile([128, CHL], f32, tag=f"ps{c}", name=f"ps{c}")
                        pss.append(ps_c)
                    if 'matmul' in ablate:
                        for c in range(NCHUNK):
                            nc.vector.tensor_copy(pss[c][:], bt[:, c * CHL:(c + 1) * CHL])
                    else:
                        for t in range(9):
                            k, l = divmod(t, 3)
                            for c in range(NCHUNK):
                                for j, src in enumerate((ra3, rb3)):
                                    rhs = src[:, CH_ROWS * c + k: CH_ROWS * c + k + CH_ROWS,
                                              l: l + W]
                                    lhsT = wt[:, t * 128 + 64 * j: t * 128 + 64 * j + 64]
                                    out_ap = pss[c][64 * j: 64 * j + 64, :]
                                    nc.tensor.matmul(out_ap, lhsT, rhs,
                                                     start=(t == 0), stop=(t == 8),
                                                     tile_position=(0, 64 * j),
                                                     skip_group_check=True)

                    for c in range(NCHUNK):
                        ps = pss[c]
                        if 'epilogue' in ablate:
                            o2 = outp.tile([128, CHL], f32, tag="o2")
                            nc.vector.tensor_copy(o2[:], ps[:])
                        else:
                            # u = A*psum + x16 (half ops) ; o = u + B ; o2 = clamp
                            u = outp.tile([128, CHL], f32, tag="u")
                            xa = ra3[0:64, CH_ROWS * c + 1: CH_ROWS * c + 1 + CH_ROWS, 1: 1 + W]
                            xb = rb3[64:128, CH_ROWS * c + 1: CH_ROWS * c + 1 + CH_ROWS, 1: 1 + W]
                            nc.vector.scalar_tensor_tensor(
                                u[0:64, :], ps[0:64, :], at[0:64], xa, ALU.mult, ALU.add)
                            nc.vector.scalar_tensor_tensor(
                                u[64:128, :], ps[64:128, :], at[64:128], xb, ALU.mult, ALU.add)
                            if 'badd' in ablate:
                                o = u
                            else:
                                o = outp.tile([128, CHL], f32, tag="o")
                                beng = nc.gpsimd if _CACHE.get('badd_gpsimd', False) else nc.vector
                                beng.tensor_tensor(o[:], u[:], bt[:, c * CHL:(c + 1) * CHL],
                                                   ALU.add)
                            if _CACHE.get('clamp_dve', False):
                                o2 = outp.tile([128, CHL], f32, tag="o2")
                                nc.vector.tensor_scalar(o2[:], o[:], 1.0, -1.0,
                                                        ALU.min, ALU.max)
                            else:
                                # clip(v,-1,1) = 1 - relu(2 - relu(v+1)), on ACT
                                a1 = outp.tile([128, CHL], f32, tag="a1")
                                nc.scalar.activation(a1[:], o[:], AF.Relu,
                                                     bias=1.0, scale=1.0)
                                a2 = outp.tile([128, CHL], f32, tag="a2")
                                nc.scalar.activation(a2[:], a1[:], AF.Relu,
                                                     bias=two[:], scale=-1.0)
                                o2 = outp.tile([128, CHL], f32, tag="o2")
                                nc.scalar.activation(o2[:], a2[:], AF.Copy,
                                                     bias=1.0, scale=-1.0)
                        if 'dma_out' not in ablate:
                            nc.sync.dma_start(y_d[p][:, c * CHL:(c + 1) * CHL], o2[:])

                if hw_loop:
                    with tc.For_i(0, hw_loop, 1):
                        _body()
                else:
                    _body()
    finally:
        tile_mod.tile_legalize = orig_legalize

    if compile:
        nc.compile()
    return nc


def _host_prep(x, shift1, shift2, weight, w1, gamma, beta, running_mean, running_var):
    x = np.asarray(x, np.float32)
    s1 = np.asarray(shift1, np.float32).reshape(C)
    s2 = np.asarray(shift2, np.float32).reshape(C)
    w = np.asarray(weight, np.float32)
    w1v = np.asarray(w1, np.float32).reshape(C)
    gamma = np.asarray(gamma, np.float32)
    beta = np.asarray(beta, np.float32)
    mean = np.asarray(running_mean, np.float32)
    var = np.asarray(running_var, np.float32)

    wb = np.sign(w).astype(np.float32)
    bs = (gamma / np.sqrt(var + BN_EPS)).astype(np.float32)
    A = (bs * (1.0 + w1v)).astype(np.float32)
    bb = (beta - mean * bs).astype(np.float32)

    G1 = np.einsum('oikl,i->okl', wb, s1)
    G2 = np.einsum('oikl,i->okl', wb, s2)
    G = bs[:, None, None] * (G1 + w1v[:, None, None] * G2)
    B = np.zeros((C, H, W), np.float32)
    hh = np.arange(H)[:, None]
    ww = np.arange(W)[None, :]
    for k in range(3):
        for l in range(3):
            m = ((hh + k - 1 >= 0) & (hh + k - 1 < H) &
                 (ww + l - 1 >= 0) & (ww + l - 1 < W)).astype(np.float32)
            B += G[:, k, l][:, None, None] * m[None]
    B += bb[:, None, None]

    # weights: lhsT[k, m] = wb[m, k, t].  Per tap t:
    #   cols 0-63  (img A): rows 0-63 = x16 wts, rows 64-127 = d16 wts (+I center)
    #   cols 64-127(img B): rows 0-63 = d16 wts (+I center), rows 64-127 = x16 wts
    wbT = wb.transpose(1, 0, 2, 3)  # [i, o, k, l]
    wtile = np.zeros((128, 9 * 128), np.float32)
    eye = np.eye(C, dtype=np.float32)
    for t in range(9):
        k, l = divmod(t, 3)
        blk = wbT[:, :, k, l]  # [i(K), o(M)]
        ident = eye if t == 4 else 0.0
        wtile[0:64, t * 128: t * 128 + 64] = blk             # img A x16
        wtile[64:128, t * 128: t * 128 + 64] = blk + ident   # img A d16
        wtile[0:64, t * 128 + 64: t * 128 + 128] = blk + ident  # img B d16
        wtile[64:128, t * 128 + 64: t * 128 + 128] = blk     # img B x16
    wtile16 = wtile.astype(np.float16)
    assert np.all(wtile16.astype(np.float32) == wtile)

    x16 = x.astype(np.float16)
    d16 = (x - x16.astype(np.float32)).astype(np.float16)

    N = x.shape[0]
    xr = np.zeros((N // 2, 2, 128, HP, WP), np.float16)
    # rhsA = [x16 imgA; d16 imgA]; rhsB = [d16 imgB; x16 imgB] (flipped)
    xr[:, 0, 0:64, 1:H + 1, 1:W + 1] = x16[0::2]
    xr[:, 0, 64:128, 1:H + 1, 1:W + 1] = d16[0::2]
    xr[:, 1, 0:64, 1:H + 1, 1:W + 1] = d16[1::2]
    xr[:, 1, 64:128, 1:H + 1, 1:W + 1] = x16[1::2]
    xr = xr.reshape(N // 2, 2, 128, PPOS)

    a128 = np.concatenate([A, A]).reshape(128, 1).astype(np.float32)
    b128 = np.concatenate([B.reshape(C, NPOS)] * 2, axis=0).astype(np.float32)
    return xr, wtile16, a128, b128


def kernel(**inputs):
    xr, wtile16, a128, b128 = _host_prep(**inputs)
    if 'nc' not in _CACHE:
        _CACHE['nc'] = _build_module()
    nc = _CACHE['nc']

    in_maps = []
    for core in range(N_CORES):
        in_maps.append({
            "xr": np.ascontiguousarray(xr[core * PAIRS:(core + 1) * PAIRS]),
            "wt": wtile16,
            "ascale": a128,
            "bfield": b128,
        })
    _CACHE['in_maps'] = in_maps
    res = bass_utils.run_bass_kernel_spmd(nc, in_maps,
                                          core_ids=list(range(N_CORES)))
    _CACHE['last_result'] = res

    N = N_CORES * IMGS
    y = np.empty((N, C, H, W), np.float32)
    for core in range(N_CORES):
        yc = res.results[core]["y"]  # [PAIRS, 128, NPOS]
        yc = yc.reshape(PAIRS * 2, C, H, W)
        y[core * IMGS:(core + 1) * IMGS] = yc
    return y



# revision 3
# speedup vs baseline: 1.5847x; 1.5847x over previous
"""Trainium2 Bass kernel for the binarized-conv BasicBlock problem.

Math restructure (exact up to fp16-grade rounding):
  wb = sign(weight)  (+-1 exactly representable in fp16)
  out = clip( A * conv(x, wb~) + B , -1, 1 )
where
  A[o]     = gamma/sqrt(var+eps) * (1 + w1[o])                (per channel)
  B[o,h,w] = bs*(conv(S1,wb) + w1*conv(S2,wb))[o,h,w] + bb[o] (batch-independent
             shift/edge field, computed on host)
  wb~      = wb + diag(1/A) on the center tap, so the conv also carries the
             residual:  A*(conv(x,wb) + x/A) = A*conv(x,wb) + x.
Precision: x is split on host as x = x16 + d16 (both fp16), so the conv is
fp32-accurate: conv(x) = conv(x16) + conv(d16) with near-exact +-1 weights
contracted as K=128 = [x16 ch; d16 ch] in one matmul stream per image.
Both the x16 and d16 blocks carry the diag(1/A) center tap, so the matmul
output already includes (x16+d16)/A = x/A.  Measured end-to-end max err
~2e-3 against the fp32 reference (threshold 2e-2).

PE mapping: per image pair, rhsA = [x16 A; d16 A], rhsB = [d16 B; x16 B].
Each tap is two K=128, M=64 matmuls on the two column halves of the PE
array; redundant LDWEIGHTS are deduplicated post-legalize so the column
halves stream concurrently with weights held stationary across the 7
row-chunks of a tap.  Epilogue per chunk is two DVE ops:
  u16 = fp16(A*psum + B16)   (scalar_tensor_tensor, PSUM read)
  o16 = clip(u16, -1, 1)     (tensor_scalar min/max, fp16 fast path)
and the output is shipped fp16 (host converts to fp32).
Sharding: batch 64 -> 8 cores x 8 images.
"""
import sys
for _p in ('/opt/trn_rl_repo',):
    if _p not in sys.path:
        sys.path.insert(0, _p)

import numpy as np
import concourse.bass as bass
import concourse.bacc as bacc
import concourse.tile as tile
import concourse.mybir as mybir
from concourse import bass_utils

BN_EPS = 1e-5
N_CORES = 8
C, H, W = 64, 56, 56
HP, WP = H + 2, W + 2           # padded spatial
NPOS = H * W                    # 3136
PPOS = HP * WP                  # 3364
IMGS = 8                        # images per core
PAIRS = IMGS // 2
CH_ROWS = 8                     # output rows per chunk
NCHUNK = H // CH_ROWS           # 7
CHL = CH_ROWS * W               # 448
WARMUP_MMS = 28                 # dummy matmuls during DMA fill to trip HAM warm

f32 = mybir.dt.float32
f16 = mybir.dt.float16
ALU = mybir.AluOpType

_CACHE = {}


def _ldw_sig(inst):
    ap = inst.ins[0]
    bap = ap.bass_ap
    return (bap.tensor.name, bap.offset, str(bap.ap), str(inst.tile_position))


def _dedup_ldweights(ordered, enable=True):
    """Drop LDWEIGHTS that reload the exact weights already resident in the
    same PE column group, so matmuls on alternating column halves stream
    back-to-back (a reload between them serializes the halves: the load
    conflicts with the in-flight matmul's row groups)."""
    if not enable:
        return ordered
    for bb, insts in ordered.items():
        last = {}     # col position -> (sig, kept name)
        remap = {}
        keep = []
        pending = None  # dropped ldw awaiting its matmul to absorb deps
        for inst in insts:
            if isinstance(inst, mybir.InstLdweights):
                sig = _ldw_sig(inst)
                col = (inst.tile_position or (0, 0))[1]
                ent = last.get(col)
                if ent is not None and ent[0] == sig:
                    remap[inst.name] = ent[1]
                    pending = inst
                    continue
                last[col] = (sig, inst.name)
            elif isinstance(inst, mybir.InstMatmult) and pending is not None:
                try:
                    inst.merge_dependencies_from(pending)
                except Exception:
                    inst.add_sync_dependencies_from(pending)
                pending = None
            keep.append(inst)
        if remap:
            for inst in keep:
                inst.remap_dependency_names(remap)
        ordered[bb] = keep
    return ordered


def _build_module(repeat=1, ablate=(), compile=True, dedup=True,
                  warmup=WARMUP_MMS):
    nc = bacc.Bacc("TRN2", target_bir_lowering=False, debug=False,
                   enable_asserts=False, num_devices=N_CORES)

    # pre-padded on host: [pair, {A,B}, 128, 58*58] fp16
    xr_d = nc.dram_tensor("xr", [PAIRS, 2, 128, PPOS], f16, kind="ExternalInput").ap()
    w_d = nc.dram_tensor("wt", [128, 9 * 128], f16, kind="ExternalInput").ap()
    a_d = nc.dram_tensor("ascale", [128, 1], f32, kind="ExternalInput").ap()
    b_d = nc.dram_tensor("bfield", [128, NPOS], f16, kind="ExternalInput").ap()
    y_d = nc.dram_tensor("y", [PAIRS, 128, NPOS], f16, kind="ExternalOutput").ap()

    import concourse.tile as tile_mod
    orig_legalize = tile_mod.tile_legalize
    tile_mod.tile_legalize = lambda ordered, nc_, _o=orig_legalize: _dedup_ldweights(
        _o(ordered, nc_), enable=dedup)
    try:
        with tile.TileContext(nc) as tc:
            with tc.tile_pool(name="const", bufs=1) as constp, \
                 tc.tile_pool(name="rhs", bufs=2) as rhsp, \
                 tc.tile_pool(name="eout", bufs=4) as outp, \
                 tc.tile_pool(name="psum", bufs=1, space="PSUM") as psp:
                # weights first: small, and needed before the first matmul
                wt = constp.tile([128, 9 * 128], f16)
                nc.sync.dma_start(wt[:], w_d[:])

                # rhs DMAs for pair 0 are issued inside the loop below; the
                # tile pool is declared before the remaining constants so
                # the framework orders pair-0 input ahead of at/bt.
                at = constp.tile([128, 1], f32)
                bt = constp.tile([128, NPOS], f16)

                # PE warmup during the DMA fill: dummy matmuls on a zeroed
                # rhs keep the HAM activity window busy so the first real
                # matmul runs at full clock.  No deps on input DMAs.
                if warmup:
                    wrm = constp.tile([128, 256], f16)
                    nc.vector.memset(wrm[:], 0.0)
                    wps = psp.tile([64, 256], f32, tag="wps", name="wps")
                    for i in range(warmup):
                        nc.tensor.matmul(wps[:], wrm[:, 0:64], wrm[:],
                                         start=True, stop=True,
                                         tile_position=(0, 0),
                                         skip_group_check=True)

                def _body():
                  first = True
                  for p in [pp for _ in range(repeat) for pp in range(PAIRS)]:
                    ra = rhsp.tile([128, PPOS], f16, tag="ra")
                    rb = rhsp.tile([128, PPOS], f16, tag="rb")
                    if 'dma_in' not in ablate:
                        nc.sync.dma_start(ra[:], xr_d[p, 0])
                        nc.sync.dma_start(rb[:], xr_d[p, 1])
                    if first:
                        # constants land after pair-0 input; they are only
                        # needed by the first epilogue (~15us later)
                        nc.sync.dma_start(at[:], a_d[:])
                        nc.sync.dma_start(bt[:], b_d[:])
                        first = False
                    ra3 = ra[:].rearrange("p (h w) -> p h w", w=WP)
                    rb3 = rb[:].rearrange("p (h w) -> p h w", w=WP)

                    pss = []
                    for c in range(NCHUNK):
                        ps_c = psp.tile([128, CHL], f32, tag=f"ps{c}", name=f"ps{c}")
                        pss.append(ps_c)
                    if 'matmul' in ablate:
                        for c in range(NCHUNK):
                            nc.vector.tensor_copy(pss[c][:], bt[:, c * CHL:(c + 1) * CHL])
                    else:
                        for t in range(9):
                            k, l = divmod(t, 3)
                            for c in range(NCHUNK):
                                for j, src in enumerate((ra3, rb3)):
                                    rhs = src[:, CH_ROWS * c + k: CH_ROWS * c + k + CH_ROWS,
                                              l: l + W]
                                    lhsT = wt[:, t * 128 + 64 * j: t * 128 + 64 * j + 64]
                                    out_ap = pss[c][64 * j: 64 * j + 64, :]
                                    nc.tensor.matmul(out_ap, lhsT, rhs,
                                                     start=(t == 0), stop=(t == 8),
                                                     tile_position=(0, 64 * j),
                                                     skip_group_check=True)

                    for c in range(NCHUNK):
                        ps = pss[c]
                        if 'epilogue' in ablate:
                            o2 = outp.tile([128, CHL], f16, tag="o2")
                            nc.vector.tensor_copy(o2[:], ps[:])
                        else:
                            # u16 = fp16(A*psum + B); o16 = clip(u16,-1,1)
                            u = outp.tile([128, CHL], f16, tag="u")
                            nc.vector.scalar_tensor_tensor(
                                u[:], ps[:], at[:], bt[:, c * CHL:(c + 1) * CHL],
                                ALU.mult, ALU.add)
                            o2 = outp.tile([128, CHL], f16, tag="o2")
                            nc.vector.tensor_scalar(o2[:], u[:], 1.0, -1.0,
                                                    ALU.min, ALU.max)
                        if 'dma_out' not in ablate:
                            nc.sync.dma_start(y_d[p][:, c * CHL:(c + 1) * CHL], o2[:])

                _body()
    finally:
        tile_mod.tile_legalize = orig_legalize

    if compile:
        nc.compile()
    return nc


def _host_prep(x, shift1, shift2, weight, w1, gamma, beta, running_mean, running_var):
    x = np.asarray(x, np.float32)
    s1 = np.asarray(shift1, np.float32).reshape(C)
    s2 = np.asarray(shift2, np.float32).reshape(C)
    w = np.asarray(weight, np.float32)
    w1v = np.asarray(w1, np.float32).reshape(C)
    gamma = np.asarray(gamma, np.float32)
    beta = np.asarray(beta, np.float32)
    mean = np.asarray(running_mean, np.float32)
    var = np.asarray(running_var, np.float32)

    wb = np.sign(w).astype(np.float32)
    bs = (gamma / np.sqrt(var + BN_EPS)).astype(np.float32)
    A = (bs * (1.0 + w1v)).astype(np.float32)
    bb = (beta - mean * bs).astype(np.float32)
    invA = (1.0 / A).astype(np.float32)

    G1 = np.einsum('oikl,i->okl', wb, s1)
    G2 = np.einsum('oikl,i->okl', wb, s2)
    G = bs[:, None, None] * (G1 + w1v[:, None, None] * G2)
    B = np.zeros((C, H, W), np.float32)
    hh = np.arange(H)[:, None]
    ww = np.arange(W)[None, :]
    for k in range(3):
        for l in range(3):
            m = ((hh + k - 1 >= 0) & (hh + k - 1 < H) &
                 (ww + l - 1 >= 0) & (ww + l - 1 < W)).astype(np.float32)
            B += G[:, k, l][:, None, None] * m[None]
    B += bb[:, None, None]

    # weights: lhsT[k, m] = wb[m, k, t].  Per tap t:
    #   cols 0-63  (img A): rows 0-63 = x16 wts, rows 64-127 = d16 wts
    #   cols 64-127(img B): rows 0-63 = d16 wts, rows 64-127 = x16 wts
    # Center tap carries diag(1/A) on BOTH the x16 and d16 rows so the
    # matmul output includes (x16+d16)/A and the BN scale restores +x.
    wbT = wb.transpose(1, 0, 2, 3)  # [i, o, k, l]
    wtile = np.zeros((128, 9 * 128), np.float32)
    identA = np.diag(invA)
    for t in range(9):
        k, l = divmod(t, 3)
        blk = wbT[:, :, k, l]  # [i(K), o(M)]
        ident = identA if t == 4 else 0.0
        wtile[0:64, t * 128: t * 128 + 64] = blk + ident         # img A x16
        wtile[64:128, t * 128: t * 128 + 64] = blk + ident       # img A d16
        wtile[0:64, t * 128 + 64: t * 128 + 128] = blk + ident   # img B d16
        wtile[64:128, t * 128 + 64: t * 128 + 128] = blk + ident # img B x16
    wtile16 = wtile.astype(np.float16)

    x16 = x.astype(np.float16)
    d16 = (x - x16.astype(np.float32)).astype(np.float16)

    N = x.shape[0]
    xr = np.zeros((N // 2, 2, 128, HP, WP), np.float16)
    # rhsA = [x16 imgA; d16 imgA]; rhsB = [d16 imgB; x16 imgB] (flipped)
    xr[:, 0, 0:64, 1:H + 1, 1:W + 1] = x16[0::2]
    xr[:, 0, 64:128, 1:H + 1, 1:W + 1] = d16[0::2]
    xr[:, 1, 0:64, 1:H + 1, 1:W + 1] = d16[1::2]
    xr[:, 1, 64:128, 1:H + 1, 1:W + 1] = x16[1::2]
    xr = xr.reshape(N // 2, 2, 128, PPOS)

    a128 = np.concatenate([A, A]).reshape(128, 1).astype(np.float32)
    b128 = np.concatenate([B.reshape(C, NPOS)] * 2, axis=0).astype(np.float16)
    return xr, wtile16, a128, b128


def kernel(**inputs):
    xr, wtile16, a128, b128 = _host_prep(**inputs)
    if 'nc' not in _CACHE:
        _CACHE['nc'] = _build_module()
    nc = _CACHE['nc']

    in_maps = []
    for core in range(N_CORES):
        in_maps.append({
            "xr": np.ascontiguousarray(xr[core * PAIRS:(core + 1) * PAIRS]),
            "wt": wtile16,
            "ascale": a128,
            "bfield": b128,
        })
    _CACHE['in_maps'] = in_maps
    res = bass_utils.run_bass_kernel_spmd(nc, in_maps,
                                          core_ids=list(range(N_CORES)))
    _CACHE['last_result'] = res

    N = N_CORES * IMGS
    y = np.empty((N, C, H, W), np.float32)
    for core in range(N_CORES):
        yc = res.results[core]["y"]  # [PAIRS, 128, NPOS] fp16
        yc = yc.astype(np.float32).reshape(PAIRS * 2, C, H, W)
        y[core * IMGS:(core + 1) * IMGS] = yc
    return y
